# revision 5
# baseline (speedup 1.0000x reference)
"""ColBERT pairwise + in-batch negative CE loss on 8 Trainium2 NeuronCores.

Problem shapes (hardcoded): B=64, N=32, S=256, D=128, fp32.

reference:
    pos_scores[b]  = sum_n max_s  q[b,n,:] . d[b,s,:]
    neg_scores[b]  = sum_n max_s  q[b,n,:] . neg[b,s,:]
    scores[b,c]    = sum_n max_s  q[b,n,:] . d[c,s,:]
    loss = (mean softplus(neg_scores - pos_scores)
            + mean softplus(max_offdiag_c scores[b,c] - scores[b,b])) / 2

Sharding: in-batch scores sharded over the doc dim c (8 docs/core; every core
sees all 64*32 query rows).  Per core the 8 local docs split into two paths
that balance the vector and scalar engines:

- docs 0-3 (straight): score tiles land in PSUM as [128 q, 4 docs x 256 s];
  the vector engine reduce_max's them directly (the only engine that can
  max-reduce, capped at 1 elem/cycle from PSUM).
- docs 4-7 (LSE): matmuls run doc-stationary so tiles land TRANSPOSED as
  [128 s, q].  The scalar engine applies exp(t*x - t*C) (same cost as a
  copy), and the tensor engine contracts the s-partitions with a one-hot
  stationary operand, accumulating sum_s exp into PSUM.  The host finishes
  with log(sum)/t + C ~= max_s (t=3, C=45: rel err ~7e-3, tolerance 2e-2).

This splits the PSUM-drain work (the kernel's true bottleneck: every score
element must leave PSUM through the vector OR scalar engine at 1 elem/cycle)
across both engines, with the tensor engine absorbing the s-sum.

The pairwise-neg term is data-parallel over b (8 queries + neg docs per
core), packed into one [128, 512] PSUM tile via tile_position.
"""

import sys

import numpy as np


def _ensure_path():
    try:
        import concourse  # noqa: F401
    except ImportError:
        sys.path.insert(0, "/opt/trn_rl_repo")


_ensure_path()

import concourse.bacc as bacc  # noqa: E402
import concourse.mybir as mybir  # noqa: E402
from concourse.bass_utils import run_bass_kernel_spmd  # noqa: E402
from concourse.tile import TileContext  # noqa: E402

B, N, S, D = 64, 32, 256, 128
NC = 8
CL = B // NC  # docs / queries per core (8)
BN = B * N  # 2048 query rows
NEG_INF_DIAG = 1000000.0

LSE_T = 3.0
LSE_C = 45.0

F32 = mybir.dt.float32
F16 = mybir.dt.float16
BF16 = mybir.dt.bfloat16
MMDT = mybir.dt.float16

_CACHE = {}


def _install_ntff_shim():
    """Best-effort: register the axon NTFF profile hook so BASS_TRACE=1
    produces hardware profiles.  Safe no-op when unavailable."""
    try:
        import types

        import antenv

        if "antenv.axon_hooks" in sys.modules:
            return
        import trn_agent_boot.trn_boot as tb

        mod = types.ModuleType("antenv.axon_hooks")
        _hook = [None]
        mod.set_axon_ntff_profile_hook = lambda h: _hook.__setitem__(0, h)
        mod.get_axon_ntff_profile_hook = lambda: _hook[0]
        sys.modules["antenv.axon_hooks"] = mod
        antenv.axon_hooks = mod
        mod.set_axon_ntff_profile_hook(
            tb._ntff_profile_via_ctypes("/opt/axon/libaxon_pjrt.so")
        )
    except Exception:
        pass


def _build():
    nc = bacc.Bacc("TRN2", target_bir_lowering=False, debug=False, num_devices=NC)
    qT = nc.dram_tensor("qT", [D, BN], MMDT, kind="ExternalInput")
    dstr = nc.dram_tensor("dstr", [D, 4 * S], MMDT, kind="ExternalInput")
    dlse = nc.dram_tensor("dlse", [D, 4 * S], MMDT, kind="ExternalInput")
    nT = nc.dram_tensor("nT", [D, CL * S], MMDT, kind="ExternalInput")
    qp = nc.dram_tensor("qp", [D, CL * N], MMDT, kind="ExternalInput")
    ones = nc.dram_tensor("ones", [D, 4], F16, kind="ExternalInput")
    ehot = nc.dram_tensor("ehot", [D, 16], BF16, kind="ExternalInput")
    epi_d = nc.dram_tensor("epi", [4, 66], F32, kind="ExternalOutput")
    lse_d = nc.dram_tensor("lse", [4, BN], F32, kind="ExternalOutput")

    X = mybir.AxisListType.X

    with TileContext(nc) as tc:
        with (
            tc.tile_pool(name="sb", bufs=1) as sb,
            tc.tile_pool(name="et", bufs=2) as etp,
            tc.tile_pool(name="pstr", bufs=1, space="PSUM") as pstr,
            tc.tile_pool(name="pt", bufs=1, space="PSUM") as ptp,
            tc.tile_pool(name="psum", bufs=1, space="PSUM") as pss,
        ):
            qs = sb.tile([D, BN], MMDT, tag="qs")
            ds = sb.tile([D, 4 * S], MMDT, tag="ds")
            dl = sb.tile([D, 4 * S], MMDT, tag="dl")
            ns = sb.tile([D, CL * S], MMDT, tag="ns")
            qps = sb.tile([D, CL * N], MMDT, tag="qps")
            onesb = sb.tile([D, 4], F16, tag="ones")
            ehotb = sb.tile([D, 16], BF16, tag="ehot")
            maxall = sb.tile([128, 66], F16, tag="maxall")
            expbias = sb.tile([128, 1], F32, tag="expbias")
            nc.gpsimd.memset(expbias[:, :], -LSE_T * LSE_C)
            lse_sb = sb.tile([4, BN], F32, tag="lse_sb")
            outsb = sb.tile([4, 66], F32, tag="outsb")

            # Input DMAs, graduated so the first work can start early.
            nc.sync.dma_start(out=qs[:, 0:256], in_=qT[:, 0:256])
            nc.sync.dma_start(out=ds[:, 0:512], in_=dstr[:, 0:512])
            nc.sync.dma_start(out=ds[:, 512:1024], in_=dstr[:, 512:1024])
            nc.sync.dma_start(out=dl[:, :], in_=dlse[:, :])
            nc.sync.dma_start(out=qs[:, 256:1024], in_=qT[:, 256:1024])
            nc.sync.dma_start(out=qs[:, 1024:2048], in_=qT[:, 1024:2048])
            for p4 in range(2):
                sl = slice(1024 * p4, 1024 * (p4 + 1))
                nc.sync.dma_start(out=ns[:, sl], in_=nT[:, sl])
            nc.sync.dma_start(out=qps[:, :], in_=qp[:, :])
            nc.sync.dma_start(out=onesb[:, :], in_=ones[:, :])
            nc.sync.dma_start(out=ehotb[:, :], in_=ehot[:, :])

            EXP = mybir.ActivationFunctionType.Exp

            # Interleave straight pairs (p = 0..7) with LSE tiles so the PE
            # never stalls on the single-buffered straight PSUM tile.
            # LSE tiles: (qphase h2, doc j, s-half sh): 16 of [128s, 1024q].
            lse_tiles = [
                (h2, j, sh) for h2 in range(2) for j in range(4) for sh in range(2)
            ]
            sum_tiles = {}
            li = 0  # next LSE tile to emit

            def emit_lse(count):
                nonlocal li
                for _ in range(count):
                    if li >= 16:
                        return
                    h2, j, sh = lse_tiles[li]
                    idx = li % 8  # accumulation index within this qphase
                    if idx == 0:
                        sum_tiles[h2] = pss.tile([4, 1024], F32, tag="sum", name=f"sum{h2}")
                    st = sum_tiles[h2]
                    T = ptp.tile([128, 1024], F32, tag="lt")
                    for u in range(2):
                        nc.tensor.matmul(
                            T[:, 512 * u : 512 * (u + 1)],
                            dl[:, 256 * j + 128 * sh : 256 * j + 128 * (sh + 1)],
                            qs[:, 1024 * h2 + 512 * u : 1024 * h2 + 512 * (u + 1)],
                            start=True,
                            stop=True,
                        )
                    et = etp.tile([128, 1024], BF16, tag="et")
                    nc.scalar.activation(
                        et[:, :], T[:, :], EXP, bias=expbias[:, :], scale=LSE_T
                    )
                    for u in range(2):
                        nc.tensor.matmul(
                            st[:, 512 * u : 512 * (u + 1)],
                            ehotb[:, 4 * j : 4 * (j + 1)],
                            et[:, 512 * u : 512 * (u + 1)],
                            start=(idx == 0),
                            stop=(idx == 7),
                        )
                    li += 1
                    if idx == 7:
                        h2d = lse_tiles[li - 1][0]
                        dst = lse_sb[:, 1024 * h2d : 1024 * (h2d + 1)]
                        if h2d == 0:
                            nc.vector.tensor_copy(dst, sum_tiles[h2d][:, :])
                        else:
                            nc.scalar.copy(dst, sum_tiles[h2d][:, :])
                        nc.sync.dma_start(
                            out=lse_d[:, 1024 * h2d : 1024 * (h2d + 1)], in_=dst
                        )

            for p in range(8):
                Sp = pstr.tile([128, 2048], F32, tag="sp")
                for h in range(2):
                    m = 2 * p + h
                    for u in range(2):
                        nc.tensor.matmul(
                            Sp[:, 1024 * h + 512 * u : 1024 * h + 512 * (u + 1)],
                            qs[:, 128 * m : 128 * (m + 1)],
                            ds[:, 512 * u : 512 * (u + 1)],
                            start=True,
                            stop=True,
                        )
                nc.vector.reduce_max(
                    maxall[:, 8 * p : 8 * p + 8],
                    Sp[:, :].rearrange("p (g s) -> p g s", s=S),
                    axis=X,
                )
                emit_lse(2)

            # Pairwise neg term: 8 small matmuls (M=32) col-packed 4-way via
            # tile_position into ONE (128, 512) region of a straight-pool tile.
            ptw = pstr.tile([128, 2048], F32, tag="sp")
            for b in range(CL):
                g, j = divmod(b, 4)
                nc.tensor.matmul(
                    ptw[32 * j : 32 * (j + 1), 256 * g : 256 * (g + 1)],
                    qps[:, 32 * b : 32 * (b + 1)],
                    ns[:, 256 * b : 256 * (b + 1)],
                    start=True,
                    stop=True,
                    tile_position=(0, 32 * j),
                )
            nc.vector.reduce_max(
                maxall[:, 64:66],
                ptw[:, 0:512].rearrange("p (g s) -> p g s", s=S),
                axis=X,
            )
            emit_lse(16)

            # n-sum via block-ones matmul: out[j, col] = sum_n maxall[32j+n, col]
            for c0, c1 in ((0, 32), (32, 66)):
                ot = ptp.tile([128, 1024], F32, tag="lt")
                nc.tensor.matmul(
                    ot[0:4, 0 : c1 - c0],
                    onesb[:, :],
                    maxall[:, c0:c1],
                    start=True,
                    stop=True,
                )
                nc.vector.tensor_copy(outsb[:, c0:c1], ot[0:4, 0 : c1 - c0])
                nc.sync.dma_start(out=epi_d[:, c0:c1], in_=outsb[:, c0:c1])

    nc.finalize()
    return nc


LAST_RESULT = None


def kernel(query_embeddings, doc_embeddings, neg_doc_embeddings):
    global LAST_RESULT
    _install_ntff_shim()

    q = np.asarray(query_embeddings, dtype=np.float32)
    d = np.asarray(doc_embeddings, dtype=np.float32)
    g = np.asarray(neg_doc_embeddings, dtype=np.float32)
    assert q.shape == (B, N, D) and d.shape == (B, S, D) and g.shape == (B, S, D)

    qT_all = np.ascontiguousarray(
        q.transpose(2, 0, 1).reshape(D, BN).astype(np.float16)
    )
    ones_blk = np.zeros((D, 4), dtype=np.float16)
    ones_blk[np.arange(D), np.arange(D) // 32] = 1.0
    import ml_dtypes

    ehot_blk = np.zeros((D, 16), dtype=ml_dtypes.bfloat16)
    for j in range(4):
        ehot_blk[:, 4 * j + j] = 1.0

    in_maps = []
    for k in range(NC):
        dk = d[CL * k : CL * (k + 1)]  # (8, S, D)
        dstr_k = np.ascontiguousarray(
            dk[0:4].transpose(2, 0, 1).reshape(D, 4 * S).astype(np.float16)
        )
        dlse_k = np.ascontiguousarray(
            dk[4:8].transpose(2, 0, 1).reshape(D, 4 * S).astype(np.float16)
        )
        nT_k = np.ascontiguousarray(
            g[CL * k : CL * (k + 1)].transpose(2, 0, 1).reshape(D, CL * S)
            .astype(np.float16)
        )
        qp_k = np.ascontiguousarray(qT_all[:, CL * N * k : CL * N * (k + 1)])
        in_maps.append(
            {
                "qT": qT_all,
                "dstr": dstr_k,
                "dlse": dlse_k,
                "nT": nT_k,
                "qp": qp_k,
                "ones": ones_blk,
                "ehot": ehot_blk,
            }
        )

    if "nc" not in _CACHE:
        _CACHE["nc"] = _build()
    res = run_bass_kernel_spmd(_CACHE["nc"], in_maps, core_ids=list(range(NC)))
    LAST_RESULT = res

    scores = np.empty((B, B), dtype=np.float64)
    negpair = np.empty((B,), dtype=np.float64)
    for k in range(NC):
        epi = res.results[k]["epi"]  # (4, 66)
        lse = res.results[k]["lse"].astype(np.float64)  # (4, 2048)
        # straight docs: col 8p+c -> chunk 2p + c//4, doc c%4; row j -> b
        for p in range(8):
            for c in range(8):
                m = 2 * p + (c // 4)
                for j in range(4):
                    scores[4 * m + j, CL * k + (c % 4)] = epi[j, 8 * p + c]
        for gcol in range(2):
            for j in range(4):
                negpair[CL * k + 4 * gcol + j] = epi[j, 64 + gcol]
        # LSE docs: sum_n log(sumexp)/t + N*C
        lse = np.maximum(lse, 1e-300)
        ln = np.log(lse) / LSE_T  # (4, 2048)
        ln = ln.reshape(4, B, N).sum(axis=2) + N * LSE_C  # (4, B)
        scores[:, CL * k + 4 : CL * k + 8] = ln.T

    pos = np.diagonal(scores)
    l1 = np.logaddexp(0.0, negpair - pos).mean()
    neg_ib = (scores - np.eye(B, dtype=np.float64) * NEG_INF_DIAG).max(axis=1)
    l2 = np.logaddexp(0.0, neg_ib - pos).mean()
    return np.asarray((l1 + l2) / 2.0, dtype=np.float32)


# revision 6
# speedup vs baseline: 1.3533x; 1.3533x over previous
"""ColBERT pairwise + in-batch negative CE loss on 8 Trainium2 NeuronCores.

Problem shapes (hardcoded): B=64, N=32, S=256, D=128, fp32.

reference:
    pos_scores[b]  = sum_n max_s  q[b,n,:] . d[b,s,:]
    neg_scores[b]  = sum_n max_s  q[b,n,:] . neg[b,s,:]
    scores[b,c]    = sum_n max_s  q[b,n,:] . d[c,s,:]
    loss = (mean softplus(neg_scores - pos_scores)
            + mean softplus(max_offdiag_c scores[b,c] - scores[b,b])) / 2

Sharding: in-batch scores sharded over the doc dim c (8 docs per core; every
core sees all 64*32 query rows).  The pairwise-neg term is data-parallel over
b.  All operands are host-pre-transposed to d-major so the contraction dim
d=128 maps onto the PE partition dim with zero device transposes.

The kernel's true bottleneck is the PSUM drain: every score element must
leave PSUM through the vector OR scalar engine at 1 elem/cycle/lane.  The 16
score chunks (each [128 q, 8 docs x 256 s] fp32 in PSUM) are therefore split
between the two engines, interleaved D,A,A,A so both drain concurrently:

- D-units (m % 4 == 0): vector engine reduce_max directly from PSUM
  (~2.1us per chunk).
- A-units: scalar engine copies PSUM -> SBUF f16 (~2.0us, 1 elem/cycle),
  and the vector engine finishes with a tensor_max halving tree, which
  runs at 2 elem/cycle in 16-bit (2x_1p) and is batched over 3 units to
  amortize per-op overhead (~1.35us per chunk).

Per core the device produces a (4, 130) fp32 tile:
  cols 0..127:  col 8*m+c, row j  ->  sum_n max_s (q[4m+j] . d_local[c])
  cols 128/129: col 128+g, row j  ->  neg_scores for local b = 4g+j
The host assembles the full (64, 64) scores matrix + the 64 neg pairwise
scores and applies the trivial softplus/mean epilogue (128 scalars).
"""

import sys

import numpy as np


def _ensure_path():
    try:
        import concourse  # noqa: F401
    except ImportError:
        sys.path.insert(0, "/opt/trn_rl_repo")


_ensure_path()

import concourse.bacc as bacc  # noqa: E402
import concourse.mybir as mybir  # noqa: E402
from concourse.bass_utils import run_bass_kernel_spmd  # noqa: E402
from concourse.tile import TileContext  # noqa: E402

B, N, S, D = 64, 32, 256, 128
NC = 8
CL = B // NC  # docs / queries per core (8)
BN = B * N  # 2048 query rows
DCOLS = CL * S  # 2048 doc columns per core
NEG_INF_DIAG = 1000000.0

F32 = mybir.dt.float32
F16 = mybir.dt.float16
MMDT = mybir.dt.float16  # dtype used by the matmul operands

_CACHE = {}


def _install_ntff_shim():
    """Best-effort: register the axon NTFF profile hook so BASS_TRACE=1
    produces hardware profiles.  Safe no-op when unavailable."""
    try:
        import types

        import antenv

        if "antenv.axon_hooks" in sys.modules:
            return
        import trn_agent_boot.trn_boot as tb

        mod = types.ModuleType("antenv.axon_hooks")
        _hook = [None]
        mod.set_axon_ntff_profile_hook = lambda h: _hook.__setitem__(0, h)
        mod.get_axon_ntff_profile_hook = lambda: _hook[0]
        sys.modules["antenv.axon_hooks"] = mod
        antenv.axon_hooks = mod
        mod.set_axon_ntff_profile_hook(
            tb._ntff_profile_via_ctypes("/opt/axon/libaxon_pjrt.so")
        )
    except Exception:
        pass


def _build():
    nc = bacc.Bacc("TRN2", target_bir_lowering=False, debug=False, num_devices=NC)
    qT = nc.dram_tensor("qT", [D, BN], MMDT, kind="ExternalInput")
    dT = nc.dram_tensor("dT", [D, DCOLS], MMDT, kind="ExternalInput")
    nT = nc.dram_tensor("nT", [D, DCOLS], MMDT, kind="ExternalInput")
    qp = nc.dram_tensor("qp", [D, CL * N], MMDT, kind="ExternalInput")
    ones = nc.dram_tensor("ones", [D, 4], F16, kind="ExternalInput")
    out_d = nc.dram_tensor("out", [4, 130], F32, kind="ExternalOutput")

    X = mybir.AxisListType.X

    with TileContext(nc) as tc:
        with (
            tc.tile_pool(name="sb", bufs=1) as sb,
            tc.tile_pool(name="ar", bufs=2) as arp,
            tc.tile_pool(name="tr", bufs=2) as trp,
            tc.tile_pool(name="ps", bufs=2, space="PSUM") as ps,
        ):
            qs = sb.tile([D, BN], MMDT, tag="qs")
            ds = sb.tile([D, DCOLS], MMDT, tag="ds")
            ns = sb.tile([D, DCOLS], MMDT, tag="ns")
            qps = sb.tile([D, CL * N], MMDT, tag="qps")
            onesb = sb.tile([D, 4], F16, tag="ones")
            maxall = sb.tile([128, 130], F16, tag="maxall")
            outsb = sb.tile([4, 130], F32, tag="outsb")

            # DMA order: graduated piece sizes so chunk m=0 (qs cols 0:128,
            # ds cols 0:1024) can start after ~300KB instead of the full load.
            nc.sync.dma_start(out=qs[:, 0:128], in_=qT[:, 0:128])
            nc.sync.dma_start(out=ds[:, 0:512], in_=dT[:, 0:512])
            nc.sync.dma_start(out=ds[:, 512:1024], in_=dT[:, 512:1024])
            nc.sync.dma_start(out=qs[:, 128:1024], in_=qT[:, 128:1024])
            nc.sync.dma_start(out=ds[:, 1024:2048], in_=dT[:, 1024:2048])
            nc.sync.dma_start(out=qs[:, 1024:2048], in_=qT[:, 1024:2048])
            for p in range(4):
                sl = slice(512 * p, 512 * (p + 1))
                nc.sync.dma_start(out=ns[:, sl], in_=nT[:, sl])
            nc.sync.dma_start(out=qps[:, :], in_=qp[:, :])
            nc.sync.dma_start(out=onesb[:, :], in_=ones[:, :])

            # In-batch term: query chunk m (128 rows) x all 2048 local doc
            # cols -> PSUM [128, 2048].  m%4==0 units are reduced directly by
            # the vector engine; the other 12 are drained by the scalar
            # engine into an f16 arena and max-tree'd by the vector engine
            # in batches of 3.
            arena = None
            for m in range(16):
                t = ps.tile([128, 2048], F32, tag="chunk")
                for u in range(4):
                    nc.tensor.matmul(
                        t[:, 512 * u : 512 * (u + 1)],
                        qs[:, 128 * m : 128 * (m + 1)],
                        ds[:, 512 * u : 512 * (u + 1)],
                        start=True,
                        stop=True,
                    )
                if m % 4 == 0:
                    nc.vector.reduce_max(
                        maxall[:, 8 * m : 8 * m + 8],
                        t[:, :].rearrange("p (g s) -> p g s", s=S),
                        axis=X,
                    )
                    continue
                slot = (m % 4) - 1  # 0..2 within the batch
                if slot == 0:
                    arena = arp.tile([128, 6144], F16, tag="arena")
                nc.scalar.copy(
                    arena[:, 2048 * slot : 2048 * (slot + 1)], t[:, :]
                )
                if slot == 2:
                    # tree over 3 units = [128, (24, 256)] f16
                    g24 = arena[:, :].rearrange("p (g s) -> p g s", s=256)
                    t1 = trp.tile([128, 3072], F16, tag="t1")
                    t1v = t1[:, :].rearrange("p (g s) -> p g s", s=128)
                    nc.vector.tensor_max(t1v, g24[:, :, 0:128], g24[:, :, 128:256])
                    t2 = trp.tile([128, 1536], F16, tag="t2")
                    t2v = t2[:, :].rearrange("p (g s) -> p g s", s=64)
                    nc.vector.tensor_max(t2v, t1v[:, :, 0:64], t1v[:, :, 64:128])
                    t3 = trp.tile([128, 768], F16, tag="t3")
                    t3v = t3[:, :].rearrange("p (g s) -> p g s", s=32)
                    nc.vector.tensor_max(t3v, t2v[:, :, 0:32], t2v[:, :, 32:64])
                    t4 = trp.tile([128, 384], F16, tag="t4")
                    t4v = t4[:, :].rearrange("p (g s) -> p g s", s=16)
                    nc.vector.tensor_max(t4v, t3v[:, :, 0:16], t3v[:, :, 16:32])
                    nc.vector.reduce_max(
                        maxall[:, 8 * (m - 2) : 8 * (m + 1)],
                        t4v,
                        axis=X,
                    )

            # Pairwise neg term: 8 small matmuls (M=32) col-packed 4-way via
            # tile_position into ONE (128, 512) region; a single segmented
            # reduce writes maxall[:, 128:130] (local b at partitions
            # 32*(b%4) + n, column 128 + b//4).
            pt = ps.tile([128, 2048], F32, tag="chunk")
            for b in range(CL):
                g, j = divmod(b, 4)
                nc.tensor.matmul(
                    pt[32 * j : 32 * (j + 1), 256 * g : 256 * (g + 1)],
                    qps[:, 32 * b : 32 * (b + 1)],
                    ns[:, 256 * b : 256 * (b + 1)],
                    start=True,
                    stop=True,
                    tile_position=(0, 32 * j),
                )
            nc.vector.reduce_max(
                maxall[:, 128:130],
                pt[:, 0:512].rearrange("p (g s) -> p g s", s=S),
                axis=X,
            )

            # n-sum via block-ones matmul: out[j, col] = sum_n maxall[32j+n, col]
            # Split at col 64 so the first half overlaps the remaining work.
            for c0, c1 in ((0, 64), (64, 130)):
                ot = ps.tile([128, 2048], F32, tag="chunk")
                nc.tensor.matmul(
                    ot[0:4, 0 : c1 - c0],
                    onesb[:, :],
                    maxall[:, c0:c1],
                    start=True,
                    stop=True,
                )
                nc.vector.tensor_copy(outsb[:, c0:c1], ot[0:4, 0 : c1 - c0])
                nc.sync.dma_start(out=out_d[:, c0:c1], in_=outsb[:, c0:c1])

    nc.finalize()
    return nc


LAST_RESULT = None


def kernel(query_embeddings, doc_embeddings, neg_doc_embeddings):
    global LAST_RESULT
    _install_ntff_shim()

    q = np.asarray(query_embeddings, dtype=np.float32)
    d = np.asarray(doc_embeddings, dtype=np.float32)
    g = np.asarray(neg_doc_embeddings, dtype=np.float32)
    assert q.shape == (B, N, D) and d.shape == (B, S, D) and g.shape == (B, S, D)

    # d-major layouts
    qT_all = np.ascontiguousarray(q.transpose(2, 0, 1).reshape(D, BN).astype(np.float16))
    ones_blk = np.zeros((D, 4), dtype=np.float16)
    ones_blk[np.arange(D), np.arange(D) // 32] = 1.0

    in_maps = []
    for k in range(NC):
        dT_k = np.ascontiguousarray(
            d[CL * k : CL * (k + 1)].transpose(2, 0, 1).reshape(D, DCOLS).astype(np.float16)
        )
        nT_k = np.ascontiguousarray(
            g[CL * k : CL * (k + 1)].transpose(2, 0, 1).reshape(D, DCOLS).astype(np.float16)
        )
        qp_k = np.ascontiguousarray(qT_all[:, CL * N * k : CL * N * (k + 1)])
        in_maps.append(
            {"qT": qT_all, "dT": dT_k, "nT": nT_k, "qp": qp_k, "ones": ones_blk}
        )

    if "nc" not in _CACHE:
        _CACHE["nc"] = _build()
    res = run_bass_kernel_spmd(_CACHE["nc"], in_maps, core_ids=list(range(NC)))
    LAST_RESULT = res

    # Assemble: scores (64, 64) and pairwise neg scores (64,)
    scores = np.empty((B, B), dtype=np.float32)
    negpair = np.empty((B,), dtype=np.float32)
    for k in range(NC):
        o = res.results[k]["out"]  # (4, 130)
        scores[:, CL * k : CL * (k + 1)] = (
            o[:, :128].reshape(4, 16, CL).transpose(1, 0, 2).reshape(B, CL)
        )
        for gcol in range(2):
            for j in range(4):
                negpair[CL * k + 4 * gcol + j] = o[j, 128 + gcol]

    pos = np.diagonal(scores).astype(np.float64)
    l1 = np.logaddexp(0.0, negpair.astype(np.float64) - pos).mean()
    neg_ib = (
        scores.astype(np.float64) - np.eye(B, dtype=np.float64) * NEG_INF_DIAG
    ).max(axis=1)
    l2 = np.logaddexp(0.0, neg_ib - pos).mean()
    return np.asarray((l1 + l2) / 2.0, dtype=np.float32)
